# revision 1
# baseline (speedup 1.0000x reference)
"""Trainium2 Bass kernel for nn_CIN: 3-layer Compressed Interaction Network.

Reference computation (per layer l, with x0 = x):
    z = einsum('bhd,bmd,ohm->bod', h, x0, W_l.reshape(o, h, m)) + b_l
    h = relu(z)
Output: concat over layers of sum_d(h)  -> (B, 384)

Strategy: data-parallel over batch across 8 cores (128 b/core). Per core,
batch is processed in 16 groups of 8 (free dim f = (b_l, d) = 512).
Each layer is a chain of K=128 matmul accumulations over chunks c (= field m):
    P_c[h, f] = h_layer[h, f] * x0[b, m=c, d]        (DVE fp16 multiply)
    z[o, f]  += W_c^T @ P_c                          (PE fp16, fp32 PSUM)
The broadcast of x0 rows across partitions is done with the DVE
STREAM_SHUFFLE front-end (mask all-zeros: lane 32q+j <- lane 32q), seeded by
a tiny 4-partition DMA. Layer 0 (h == x0, 32 fields) is remapped to full
128-partition chunks via k = (c', q, j) <-> (m = 4c'+q, m' = j).
"""

import os
import sys

import numpy as np

for _p in ("/opt/trn_rl_repo", "/root/.axon_site/_ro/trn_rl_repo"):
    if os.path.isdir(_p) and _p not in sys.path:
        sys.path.append(_p)

import concourse.bass as bass  # noqa: E402
import concourse.mybir as mybir  # noqa: E402
import concourse.tile as tile  # noqa: E402
from concourse import bacc  # noqa: E402
from concourse.bass_utils import run_bass_kernel_spmd  # noqa: E402

# Problem dims (hardcoded per spec)
B, F, D = 1024, 32, 64
H = 128  # hidden per layer
NCORES = 8
BC = B // NCORES       # 128 batch per core
GB = 8                 # batch elems per group
NG = BC // GB          # 16 groups
FREE = GB * D          # 512 moving free dim
NL = 3                 # layers

F16 = mybir.dt.float16
F32 = mybir.dt.float32

_BCAST_MASK = [0] * 32


def build_program():
    nc = bacc.Bacc("TRN2", target_bir_lowering=False)

    xrep_d = nc.dram_tensor("xrep", [NG, 128, FREE], F16, kind="ExternalInput")
    xsrca_d = nc.dram_tensor("xsrca", [NG, 4, F * FREE], F16, kind="ExternalInput")
    xsrcb_d = nc.dram_tensor("xsrcb", [NG, 4, 8 * FREE], F16, kind="ExternalInput")
    w0_d = nc.dram_tensor("w0", [128, 8, 128], F16, kind="ExternalInput")
    w1_d = nc.dram_tensor("w1", [128, F, 128], F16, kind="ExternalInput")
    w2_d = nc.dram_tensor("w2", [128, F, 128], F16, kind="ExternalInput")
    b0_d = nc.dram_tensor("b0", [128, 1], F32, kind="ExternalInput")
    b1_d = nc.dram_tensor("b1", [128, 1], F32, kind="ExternalInput")
    b2_d = nc.dram_tensor("b2", [128, 1], F32, kind="ExternalInput")
    out_d = nc.dram_tensor("outy", [128, NL, NG, GB], F32, kind="ExternalOutput")

    with tile.TileContext(nc) as tc:
        with (
            tc.tile_pool(name="singles", bufs=1) as singles,
            tc.tile_pool(name="x0b", bufs=2) as x0b_pool,
            tc.tile_pool(name="upool", bufs=2) as u_pool,
            tc.tile_pool(name="xrep", bufs=2) as xrep_pool,
            tc.tile_pool(name="ppool", bufs=4) as p_pool,
            tc.tile_pool(name="hpool", bufs=6) as h_pool,
            tc.tile_pool(name="zpool", bufs=4, space="PSUM") as z_pool,
        ):
            w0_sb = singles.tile([128, 8, 128], F16)
            w1_sb = singles.tile([128, F, 128], F16)
            w2_sb = singles.tile([128, F, 128], F16)
            b0_sb = singles.tile([128, 1], F32)
            b1_sb = singles.tile([128, 1], F32)
            b2_sb = singles.tile([128, 1], F32)
            outstage = singles.tile([128, NL, NG, GB], F32)
            nc.sync.dma_start(out=w0_sb[:], in_=w0_d[:])
            nc.sync.dma_start(out=w1_sb[:], in_=w1_d[:])
            nc.sync.dma_start(out=w2_sb[:], in_=w2_d[:])
            nc.sync.dma_start(out=b0_sb[:], in_=b0_d[:])
            nc.sync.dma_start(out=b1_sb[:], in_=b1_d[:])
            nc.sync.dma_start(out=b2_sb[:], in_=b2_d[:])

            w_views = [w0_sb, w1_sb, w2_sb]
            b_views = [b0_sb, b1_sb, b2_sb]

            def prepare(g):
                """DMA group inputs and build broadcast buffers."""
                xrep_t = xrep_pool.tile([128, FREE], F16, tag="xrep")
                nc.sync.dma_start(out=xrep_t[:], in_=xrep_d[g])
                # X0B: seed partitions {0,32,64,96} then in-place shuffle
                x0b_t = x0b_pool.tile([128, F, FREE], F16, tag="x0b")
                seed_a = x0b_t.rearrange("(q r) c f -> q r c f", q=4)[:, 0]
                nc.sync.dma_start(
                    out=seed_a, in_=xsrca_d[g].rearrange("q (c f) -> q c f", c=F)
                )
                nc.vector.stream_shuffle(
                    x0b_t.rearrange("p c f -> p (c f)"),
                    x0b_t.rearrange("p c f -> p (c f)"),
                    _BCAST_MASK,
                )
                # U (layer-0 broadcast): same trick, 8 chunks
                u_t = u_pool.tile([128, 8, FREE], F16, tag="u")
                seed_b = u_t.rearrange("(q r) c f -> q r c f", q=4)[:, 0]
                nc.sync.dma_start(
                    out=seed_b, in_=xsrcb_d[g].rearrange("q (c f) -> q c f", c=8)
                )
                nc.vector.stream_shuffle(
                    u_t.rearrange("p c f -> p (c f)"),
                    u_t.rearrange("p c f -> p (c f)"),
                    _BCAST_MASK,
                )
                return xrep_t, x0b_t, u_t

            def layer(g, l, src_h, bcast, nchunks):
                """One CIN layer for group g; returns relu'd hidden (fp16)."""
                z_t = z_pool.tile([128, FREE], F32, tag="z")
                for c in range(nchunks):
                    p_t = p_pool.tile([128, FREE], F16, tag="p")
                    nc.vector.tensor_mul(p_t[:], src_h[:], bcast[:, c])
                    nc.tensor.matmul(
                        z_t[:],
                        w_views[l][:, c] if l > 0 else w_views[0][:, c],
                        p_t[:],
                        start=(c == 0),
                        stop=(c == nchunks - 1),
                    )
                h_t = h_pool.tile([128, FREE], F16, tag="h")
                nc.scalar.activation(
                    h_t[:], z_t[:], mybir.ActivationFunctionType.Relu,
                    bias=b_views[l][:],
                )
                nc.vector.reduce_sum(
                    out=outstage[:, l, g],
                    in_=h_t.rearrange("p (b d) -> p b d", b=GB),
                    axis=mybir.AxisListType.X,
                )
                return h_t

            def do_group_layers(g, prep):
                xrep_t, x0b_t, u_t = prep
                h1 = layer(g, 0, xrep_t, u_t, 8)
                h2 = layer(g, 1, h1, x0b_t, F)
                layer(g, 2, h2, x0b_t, F)

            # process groups in pairs, layers interleaved, to hide the
            # serial mult->matmul->relu dependency at layer boundaries
            for t in range(NG // 2):
                ga, gb = 2 * t, 2 * t + 1
                pa = prepare(ga)
                pb = prepare(gb)
                h1a = layer(ga, 0, pa[0], pa[2], 8)
                h1b = layer(gb, 0, pb[0], pb[2], 8)
                h2a = layer(ga, 1, h1a, pa[1], F)
                h2b = layer(gb, 1, h1b, pb[1], F)
                layer(ga, 2, h2a, pa[1], F)
                layer(gb, 2, h2b, pb[1], F)

            nc.sync.dma_start(out=out_d[:], in_=outstage[:])

    nc.finalize()
    return nc


def host_prep(x, W0, b0, W1, b1, W2, b2):
    """Build per-core input maps (numpy only)."""
    x = np.asarray(x, dtype=np.float32)
    assert x.shape == (B, F, D), x.shape
    xh = x.astype(np.float16)

    # weights: lhsT layouts
    Wr0 = np.asarray(W0, dtype=np.float32).reshape(H, F, F)      # (o, m', m)
    t = Wr0.transpose(1, 2, 0)                                   # (m'=j, m, o)
    t = t.reshape(F, 8, 4, H).transpose(2, 0, 1, 3)              # (q, j, c', o)
    w0l = np.ascontiguousarray(t.reshape(128, 8, H)).astype(np.float16)

    Wr1 = np.asarray(W1, dtype=np.float32).reshape(H, H, F)      # (o, h, m)
    w1l = np.ascontiguousarray(Wr1.transpose(1, 2, 0)).astype(np.float16)
    Wr2 = np.asarray(W2, dtype=np.float32).reshape(H, H, F)
    w2l = np.ascontiguousarray(Wr2.transpose(1, 2, 0)).astype(np.float16)

    b0c = np.asarray(b0, dtype=np.float32).reshape(128, 1)
    b1c = np.asarray(b1, dtype=np.float32).reshape(128, 1)
    b2c = np.asarray(b2, dtype=np.float32).reshape(128, 1)

    in_maps = []
    for i in range(NCORES):
        s = xh[i * BC:(i + 1) * BC].reshape(NG, GB, F, D)        # (g, b, m, d)
        base = np.ascontiguousarray(s.transpose(0, 2, 1, 3)).reshape(NG, F, FREE)
        # xrep[g, 32q+j, f] = x[b, j, d]
        xrep = np.tile(base, (1, 4, 1))                          # (NG, 128, FREE)
        # xsrca[g, q, c*FREE + f] = x[b, c, d]
        xsrca = np.broadcast_to(
            base.reshape(NG, 1, F * FREE), (NG, 4, F * FREE)
        ).copy()
        # xsrcb[g, q, c'*FREE + f] = x[b, 4c'+q, d]
        xsrcb = np.ascontiguousarray(
            base.reshape(NG, 8, 4, FREE).transpose(0, 2, 1, 3)
        ).reshape(NG, 4, 8 * FREE)
        in_maps.append({
            "xrep": np.ascontiguousarray(xrep),
            "xsrca": xsrca,
            "xsrcb": xsrcb,
            "w0": w0l, "w1": w1l, "w2": w2l,
            "b0": b0c, "b1": b1c, "b2": b2c,
        })
    return in_maps


_NC_CACHE = {}


def _get_nc():
    if "nc" not in _NC_CACHE:
        _NC_CACHE["nc"] = build_program()
    return _NC_CACHE["nc"]


def kernel(x, W0, b0, W1, b1, W2, b2, _trace=False):
    in_maps = host_prep(x, W0, b0, W1, b1, W2, b2)
    nc = _get_nc()
    res = run_bass_kernel_spmd(nc, in_maps, list(range(NCORES)), trace=_trace)
    outs = []
    for i in range(NCORES):
        o = res.results[i]["outy"]                               # (128, 3, 16, 8)
        outs.append(o.transpose(2, 3, 1, 0).reshape(BC, NL * 128))
    full = np.concatenate(outs, axis=0).astype(np.float32)
    if _trace:
        return full, res
    return full


# revision 9
# speedup vs baseline: 18.3843x; 18.3843x over previous
"""Trainium2 Bass kernel for nn_CIN: 3-layer Compressed Interaction Network.

Reference computation (per layer l, with x0 = x):
    z = einsum('bhd,bmd,ohm->bod', h, x0, W_l.reshape(o, h, m)) + b_l
    h = relu(z)
Output: concat over layers of sum_d(h)  -> (B, 384)

Strategy: data-parallel over batch across 8 cores (128 b/core). Per core,
batch is processed in 16 groups of 8 (free dim f = (b_l, d) = 512).
Each layer is a chain of K=128 matmul accumulations over chunks c (= field m):
    P_c[h, f] = h_layer[h, f] * x0[b, m=c, d]        (DVE fp16 multiply)
    z[o, f]  += W_c^T @ P_c                          (PE fp16, fp32 PSUM)
The broadcast of x0 rows across partitions is done with the DVE
STREAM_SHUFFLE front-end (mask all-zeros: lane 32q+j <- lane 32q), seeded by
a tiny 4-partition DMA. Layer 0 (h == x0, 32 fields) is remapped to full
128-partition chunks via k = (c', q, j) <-> (m = 4c'+q, m' = j).
"""

import os
import sys

import numpy as np

for _p in ("/opt/trn_rl_repo", "/root/.axon_site/_ro/trn_rl_repo"):
    if os.path.isdir(_p) and _p not in sys.path:
        sys.path.append(_p)

import concourse.bass as bass  # noqa: E402
import concourse.mybir as mybir  # noqa: E402
import concourse.tile as tile  # noqa: E402
from concourse import bacc  # noqa: E402
from concourse.bass_utils import run_bass_kernel_spmd  # noqa: E402

# Problem dims (hardcoded per spec)
B, F, D = 1024, 32, 64
H = 128  # hidden per layer
NCORES = 8
BC = B // NCORES       # 128 batch per core
GB = 8                 # batch elems per group
NG = BC // GB          # 16 groups
FREE = GB * D          # 512 moving free dim
NL = 3                 # layers

F16 = mybir.dt.float16
F32 = mybir.dt.float32

_BCAST_MASK = [0] * 32


def build_program(repeat=1):
    nc = bacc.Bacc("TRN2", target_bir_lowering=False)

    xrep_d = nc.dram_tensor("xrep", [NG, 128, FREE], F16, kind="ExternalInput")
    xbase_d = nc.dram_tensor("xbase", [NG, F, FREE], F16, kind="ExternalInput")
    xsrcb_d = nc.dram_tensor("xsrcb", [NG, 4, 8 * FREE], F16, kind="ExternalInput")
    w0_d = nc.dram_tensor("w0", [128, 8, 128], F16, kind="ExternalInput")
    w1_d = nc.dram_tensor("w1", [128, F, 128], F16, kind="ExternalInput")
    w2_d = nc.dram_tensor("w2", [128, F, 128], F16, kind="ExternalInput")
    b0_d = nc.dram_tensor("b0", [128, 1], F32, kind="ExternalInput")
    b1_d = nc.dram_tensor("b1", [128, 1], F32, kind="ExternalInput")
    b2_d = nc.dram_tensor("b2", [128, 1], F32, kind="ExternalInput")
    out_d = nc.dram_tensor("outy", [128, NL, NG, GB], F32, kind="ExternalOutput")

    with tile.TileContext(nc) as tc:
        with (
            tc.tile_pool(name="singles", bufs=1) as singles,
            tc.tile_pool(name="x0b", bufs=2) as x0b_pool,
            tc.tile_pool(name="upool", bufs=2) as u_pool,
            tc.tile_pool(name="xrep", bufs=2) as xrep_pool,
            tc.tile_pool(name="ppool", bufs=4) as p_pool,
            tc.tile_pool(name="hpool", bufs=6) as h_pool,
            tc.tile_pool(name="zpool", bufs=4, space="PSUM") as z_pool,
        ):
            w0_sb = singles.tile([128, 8, 128], F16)
            w1_sb = singles.tile([128, F, 128], F16)
            w2_sb = singles.tile([128, F, 128], F16)
            b0_sb = singles.tile([128, 1], F32)
            b1_sb = singles.tile([128, 1], F32)
            b2_sb = singles.tile([128, 1], F32)
            outstage = singles.tile([128, NL, NG, GB], F32)
            nc.sync.dma_start(out=w0_sb[:], in_=w0_d[:])
            nc.sync.dma_start(out=w1_sb[:], in_=w1_d[:])
            nc.sync.dma_start(out=w2_sb[:], in_=w2_d[:])
            nc.sync.dma_start(out=b0_sb[:], in_=b0_d[:])
            nc.sync.dma_start(out=b1_sb[:], in_=b1_d[:])
            nc.sync.dma_start(out=b2_sb[:], in_=b2_d[:])

            w_views = [w0_sb, w1_sb, w2_sb]
            b_views = [b0_sb, b1_sb, b2_sb]

            def prepare(g):
                """DMA group inputs and build broadcast buffers (SWDGE
                partition-stride-0 replication)."""
                xrep_t = xrep_pool.tile([128, FREE], F16, tag="xrep")
                nc.sync.dma_start(out=xrep_t[:], in_=xrep_d[g])
                x0b_t = x0b_pool.tile([128, F, FREE], F16, tag="x0b")
                nc.sync.dma_start(
                    out=x0b_t[:], in_=xbase_d[g].partition_broadcast(128)
                )
                # U (layer-0 broadcast): quadrant q holds rows m = 4c'+q
                u_t = u_pool.tile([128, 8, FREE], F16, tag="u")
                for q in range(4):
                    nc.scalar.dma_start(
                        out=u_t[32 * q:32 * (q + 1)],
                        in_=xsrcb_d[g, q]
                        .rearrange("(c f) -> c f", c=8)
                        .partition_broadcast(32),
                    )
                return xrep_t, x0b_t, u_t

            MF = 8  # chunks fused per DVE multiply

            def layer(g, l, src_h, bcast, nchunks):
                """One CIN layer for group g; returns relu'd hidden (fp16)."""
                z_t = z_pool.tile([128, FREE], F32, tag="z")
                sh = src_h[:]
                sh_b = bass.AP(
                    tensor=sh.tensor, offset=sh.offset,
                    ap=[list(sh.ap[0]), [0, MF], list(sh.ap[1])],
                )
                for t0 in range(0, nchunks, MF):
                    p_t = p_pool.tile([128, MF, FREE], F16, tag="p")
                    nc.vector.tensor_mul(p_t[:], sh_b, bcast[:, t0:t0 + MF])
                    for i in range(MF):
                        c = t0 + i
                        nc.tensor.matmul(
                            z_t[:],
                            w_views[l][:, c],
                            p_t[:, i],
                            start=(c == 0),
                            stop=(c == nchunks - 1),
                        )
                h_t = h_pool.tile([128, FREE], F16, tag="h")
                nc.scalar.activation(
                    h_t[:], z_t[:], mybir.ActivationFunctionType.Relu,
                    bias=b_views[l][:],
                )
                nc.vector.reduce_sum(
                    out=outstage[:, l, g],
                    in_=h_t.rearrange("p (b d) -> p b d", b=GB),
                    axis=mybir.AxisListType.X,
                )
                return h_t

            def do_group_layers(g, prep):
                xrep_t, x0b_t, u_t = prep
                h1 = layer(g, 0, xrep_t, u_t, 8)
                h2 = layer(g, 1, h1, x0b_t, F)
                layer(g, 2, h2, x0b_t, F)

            # process groups in pairs, layers interleaved, to hide the
            # serial mult->matmul->relu dependency at layer boundaries
            for _rep in range(repeat):
                for t in range(NG // 2):
                    ga, gb = 2 * t, 2 * t + 1
                    pa = prepare(ga)
                    pb = prepare(gb)
                    h1a = layer(ga, 0, pa[0], pa[2], 8)
                    h1b = layer(gb, 0, pb[0], pb[2], 8)
                    h2a = layer(ga, 1, h1a, pa[1], F)
                    h2b = layer(gb, 1, h1b, pb[1], F)
                    layer(ga, 2, h2a, pa[1], F)
                    layer(gb, 2, h2b, pb[1], F)

                nc.sync.dma_start(out=out_d[:], in_=outstage[:])

    nc.finalize()
    return nc


def host_prep(x, W0, b0, W1, b1, W2, b2):
    """Build per-core input maps (numpy only)."""
    x = np.asarray(x, dtype=np.float32)
    assert x.shape == (B, F, D), x.shape
    xh = x.astype(np.float16)

    # weights: lhsT layouts
    Wr0 = np.asarray(W0, dtype=np.float32).reshape(H, F, F)      # (o, m', m)
    t = Wr0.transpose(1, 2, 0)                                   # (m'=j, m, o)
    t = t.reshape(F, 8, 4, H).transpose(2, 0, 1, 3)              # (q, j, c', o)
    w0l = np.ascontiguousarray(t.reshape(128, 8, H)).astype(np.float16)

    Wr1 = np.asarray(W1, dtype=np.float32).reshape(H, H, F)      # (o, h, m)
    w1l = np.ascontiguousarray(Wr1.transpose(1, 2, 0)).astype(np.float16)
    Wr2 = np.asarray(W2, dtype=np.float32).reshape(H, H, F)
    w2l = np.ascontiguousarray(Wr2.transpose(1, 2, 0)).astype(np.float16)

    b0c = np.asarray(b0, dtype=np.float32).reshape(128, 1)
    b1c = np.asarray(b1, dtype=np.float32).reshape(128, 1)
    b2c = np.asarray(b2, dtype=np.float32).reshape(128, 1)

    in_maps = []
    for i in range(NCORES):
        s = xh[i * BC:(i + 1) * BC].reshape(NG, GB, F, D)        # (g, b, m, d)
        base = np.ascontiguousarray(s.transpose(0, 2, 1, 3)).reshape(NG, F, FREE)
        # xrep[g, 32q+j, f] = x[b, j, d]
        xrep = np.tile(base, (1, 4, 1))                          # (NG, 128, FREE)
        # xsrcb[g, q, c'*FREE + f] = x[b, 4c'+q, d]
        xsrcb = np.ascontiguousarray(
            base.reshape(NG, 8, 4, FREE).transpose(0, 2, 1, 3)
        ).reshape(NG, 4, 8 * FREE)
        in_maps.append({
            "xrep": np.ascontiguousarray(xrep),
            "xbase": np.ascontiguousarray(base),
            "xsrcb": xsrcb,
            "w0": w0l, "w1": w1l, "w2": w2l,
            "b0": b0c, "b1": b1c, "b2": b2c,
        })
    return in_maps


_NC_CACHE = {}


def _get_nc():
    if "nc" not in _NC_CACHE:
        _NC_CACHE["nc"] = build_program()
    return _NC_CACHE["nc"]


def kernel(x, W0, b0, W1, b1, W2, b2, _trace=False):
    in_maps = host_prep(x, W0, b0, W1, b1, W2, b2)
    nc = _get_nc()
    res = run_bass_kernel_spmd(nc, in_maps, list(range(NCORES)), trace=_trace)
    outs = []
    for i in range(NCORES):
        o = res.results[i]["outy"]                               # (128, 3, 16, 8)
        outs.append(o.transpose(2, 3, 1, 0).reshape(BC, NL * 128))
    full = np.concatenate(outs, axis=0).astype(np.float32)
    if _trace:
        return full, res
    return full
